# revision 8
# baseline (speedup 1.0000x reference)
"""Trainium2 Bass kernel for nn_EnsembleMultiTaskDecoder.

Sharding: 8 cores = 2 ensemble-member groups x 4 batch shards (B=64 -> 16/core).
Everything is batch-local per core except a per-step 2-rank AllGather that
exchanges the two members' logits between paired cores (c <-> c+4).

Per step on each core (member e, batch rows bs..bs+16):
  a2 = h1 @ Wdec ; relu(att1T + a2T) ; scores = v . relu ; alpha = softmax
  awe = alpha @ mem (fp16 stream) ; gate = sigmoid(h1 @ Wfb + bfb)
  x = [emb[tok], gate*awe] ; 2-layer LSTM (fp32, W_ih1/W_hh1 streamed from HBM)
  logits = h2 @ W_fc + b_fc ; AllGather pair logits ; argmax(l0+l1) -> token

The 256-step loop runs as NCHUNK calls of one compiled NSTEP-step NEFF with
LSTM state carried in DRAM. Score is reconstructed on the host from per-step
stats (exact fp32 op order of the reference).
"""
import numpy as np

PAD, SOS, EOS = 0, 1, 2
MAX_LEN = 256
E, B, L, Denc, Demb, Datt, Dh, V = 2, 64, 576, 1024, 256, 256, 512, 1000
NCORES = 8
BL = 16            # batch rows per core
NSTEP = 16         # steps per NEFF call
LC = 5             # l chunks (576 = 4*128 + 64)
LCS = [128, 128, 128, 128, 64]
KX = 14            # x/h1 contraction chunks (1792 = 14*128)
BIG = 1.0e9

_compiled = None


def _build():
    import concourse.bass as bass
    import concourse.bacc as bacc
    import concourse.mybir as mybir
    import concourse.tile as tile

    f32 = mybir.dt.float32
    f16 = mybir.dt.float16
    i32 = mybir.dt.int32
    AF = mybir.ActivationFunctionType
    OP = mybir.AluOpType

    nc = bacc.Bacc("TRN2", target_bir_lowering=False, debug=False,
                   num_devices=NCORES)

    def din(name, shape, dt=f32):
        return nc.dram_tensor(name, shape, dt, kind="ExternalInput")

    def dout(name, shape, dt=f32):
        return nc.dram_tensor(name, shape, dt, kind="ExternalOutput")

    # ---- DRAM inputs (per-core slices prepared on host) ----
    mem16_d = din("mem16", [LC, 128, BL * Denc], f16)   # [lc][l][b*1024+d]
    att1T_d = din("att1T", [BL * 2, 128, L], f16)       # [(b,ac)][ap][l]
    emb_d = din("embw", [V, Demb])
    W1_d = din("W1", [KX, 128, 4 * Dh])                 # [Wih1;Whh1] chunks
    W2_d = din("W2", [8, 128, 4 * Dh])                  # [Wih2;Whh2] chunks
    Wfc_d = din("Wfc", [4, 128, V])
    Wdec_d = din("Wdec", [4, 128, Datt])
    Wfb_d = din("Wfb", [4, 128, Denc])
    vpad_d = din("vpad", [128, 2 * BL * BL], f16)       # v col-padded tiles
    b1r_d = din("b1r", [BL, 4 * Dh])
    b2r_d = din("b2r", [BL, 4 * Dh])
    bfbr_d = din("bfbr", [BL, Denc])
    bfcr_d = din("bfcr", [BL, V])
    iota_d = din("iotar", [BL, V])                      # 0..999 replicated
    ident_d = din("ident", [128, 128])                  # for PE transpose
    h1_d = din("h1_in", [BL, Dh])
    c1_d = din("c1_in", [BL, Dh])
    h2_d = din("h2_in", [BL, Dh])
    c2_d = din("c2_in", [BL, Dh])
    tok_d = din("tok_in", [BL, 1])                      # fp32 token ids
    fin_d = din("fin_in", [BL, 1])                      # fp32 0/1

    h1_o = dout("h1_out", [BL, Dh])
    c1_o = dout("c1_out", [BL, Dh])
    h2_o = dout("h2_out", [BL, Dh])
    c2_o = dout("c2_out", [BL, Dh])
    tok_o = dout("tok_out", [BL, 1])
    fin_o = dout("fin_out", [BL, 1])
    preds_o = dout("preds", [BL, NSTEP])
    m0_o = dout("m0s", [BL, NSTEP])     # stores -max(logits_e)
    m1_o = dout("m1s", [BL, NSTEP])
    s0_o = dout("s0s", [BL, NSTEP])     # sumexp(logits_e - max)
    s1_o = dout("s1s", [BL, NSTEP])
    l0v_o = dout("l0vs", [BL, NSTEP])   # logits_e[nxt]
    l1v_o = dout("l1vs", [BL, NSTEP])

    GROUPS = [[0, 4], [1, 5], [2, 6], [3, 7]]

    with tile.TileContext(nc) as tc:
        with (
            tc.tile_pool(name="res", bufs=1) as res,
            tc.tile_pool(name="stat", bufs=1) as statp,
            tc.tile_pool(name="sb", bufs=1) as sb,
            tc.tile_pool(name="mem", bufs=3) as memp,
            tc.tile_pool(name="w1", bufs=3) as w1p,
            tc.tile_pool(name="rl", bufs=4) as rlp,
            tc.tile_pool(name="ps", bufs=6, space="PSUM") as ps,
            tc.tile_pool(name="tp", bufs=2, space="PSUM") as tpp,
            tc.tile_pool(name="dram", bufs=2, space="DRAM") as dram,
        ):
            # ---- resident tiles (loaded once) ----
            Wdec = res.tile([128, 4 * Datt], f32, tag="Wdec")
            for k in range(4):
                nc.sync.dma_start(Wdec[:, k * Datt:(k + 1) * Datt], Wdec_d[k])
            Wfb = res.tile([128, 4 * Denc], f32, tag="Wfb")
            for k in range(4):
                nc.sync.dma_start(Wfb[:, k * Denc:(k + 1) * Denc], Wfb_d[k])
            vpad = res.tile([128, 2 * BL * BL], f16, tag="vpad")
            nc.sync.dma_start(vpad[:], vpad_d[:])
            b1r = res.tile([BL, 4 * Dh], f32, tag="b1r")
            nc.sync.dma_start(b1r[:], b1r_d[:])
            b2r = res.tile([BL, 4 * Dh], f32, tag="b2r")
            nc.sync.dma_start(b2r[:], b2r_d[:])
            bfbr = res.tile([BL, Denc], f32, tag="bfbr")
            nc.sync.dma_start(bfbr[:], bfbr_d[:])
            bfcr = res.tile([BL, V], f32, tag="bfcr")
            nc.sync.dma_start(bfcr[:], bfcr_d[:])
            iotar = res.tile([BL, V], f32, tag="iotar")
            nc.sync.dma_start(iotar[:], iota_d[:])
            ident = res.tile([128, 128], f32, tag="ident")
            nc.sync.dma_start(ident[:], ident_d[:])

            # ---- state tiles ----
            h1 = statp.tile([BL, Dh], f32, tag="h1")
            c1 = statp.tile([BL, Dh], f32, tag="c1")
            h2 = statp.tile([BL, Dh], f32, tag="h2")
            c2 = statp.tile([BL, Dh], f32, tag="c2")
            fin = statp.tile([BL, 1], f32, tag="fin")
            nc.sync.dma_start(h1[:], h1_d[:])
            nc.sync.dma_start(c1[:], c1_d[:])
            nc.sync.dma_start(h2[:], h2_d[:])
            nc.sync.dma_start(c2[:], c2_d[:])
            nc.sync.dma_start(fin[:], fin_d[:])
            toki = statp.tile([BL, 1], i32, tag="toki")
            tokf0 = statp.tile([BL, 1], f32, tag="tokf0")
            nc.sync.dma_start(tokf0[:], tok_d[:])
            nc.vector.tensor_copy(toki[:], tokf0[:])
            # stat accumulators
            preds_t = statp.tile([BL, NSTEP], f32, tag="preds")
            m0_t = statp.tile([BL, NSTEP], f32, tag="m0")
            m1_t = statp.tile([BL, NSTEP], f32, tag="m1")
            s0_t = statp.tile([BL, NSTEP], f32, tag="s0")
            s1_t = statp.tile([BL, NSTEP], f32, tag="s1")
            l0v_t = statp.tile([BL, NSTEP], f32, tag="l0v")
            l1v_t = statp.tile([BL, NSTEP], f32, tag="l1v")
            # xT: stationary [128, 14*16]: chunks 0-1 e, 2-9 ga, 10-13 h1
            xT = statp.tile([128, KX * BL], f32, tag="xT")
            h2T = statp.tile([128, 4 * BL], f32, tag="h2T")

            def transpose_into(dst, src_ap, ncol=BL):
                """src [p, f] -> PE transpose -> copy [f, p] into dst."""
                p, f = src_ap.shape[0], src_ap.shape[1]
                t = tpp.tile([128, BL], f32, tag="tp")
                nc.tensor.transpose(t[:f, :p], src_ap, ident[:p, :p])
                nc.vector.tensor_copy(dst, t[:f, :p])

            # fill h1T (xT chunks 10-13) and h2T from initial state
            for k in range(4):
                transpose_into(xT[:, (10 + k) * BL:(11 + k) * BL],
                               h1[:, k * 128:(k + 1) * 128])
                transpose_into(h2T[:, k * BL:(k + 1) * BL],
                               h2[:, k * 128:(k + 1) * 128])
            # initial embedding gather
            e_sb = statp.tile([BL, Demb], f32, tag="e_sb")
            nc.gpsimd.indirect_dma_start(
                out=e_sb[:], out_offset=None, in_=emb_d[:],
                in_offset=bass.IndirectOffsetOnAxis(ap=toki[:, :1], axis=0))
            for k in range(2):
                transpose_into(xT[:, k * BL:(k + 1) * BL],
                               e_sb[:, k * 128:(k + 1) * 128])

            for t in range(NSTEP):
                # ---- a2 = h1 @ Wdec -> a2T [256,16] ----
                a2_ps = ps.tile([BL, 512], f32, tag="ps")
                for k in range(4):
                    nc.tensor.matmul(a2_ps[:, :Datt],
                                     xT[:, (10 + k) * BL:(11 + k) * BL],
                                     Wdec[:, k * Datt:(k + 1) * Datt],
                                     start=(k == 0), stop=(k == 3))
                a2_sb = sb.tile([BL, Datt], f32, tag="a2_sb")
                nc.vector.tensor_copy(a2_sb[:], a2_ps[:BL, :Datt])
                a2T = sb.tile([128, 2 * BL], f32, tag="a2T")
                for ac in range(2):
                    transpose_into(a2T[:, ac * BL:(ac + 1) * BL],
                                   a2_sb[:, ac * 128:(ac + 1) * 128])

                # ---- relu(att1T + a2T) and scores ----
                sc_lo = ps.tile([BL, 512], f32, tag="ps")
                sc_hi = ps.tile([BL, 512], f32, tag="ps")
                for b in range(BL):
                    for ac in range(2):
                        a1 = rlp.tile([128, L], f16, tag="a1")
                        nc.sync.dma_start(a1[:], att1T_d[b * 2 + ac])
                        rl = rlp.tile([128, L], f16, tag="rl")
                        nc.scalar.activation(
                            rl[:], a1[:],
                            AF.Relu, bias=a2T[:, ac * BL + b:ac * BL + b + 1])
                        lhs = vpad[:, (ac * BL + b) * BL:(ac * BL + b + 1) * BL]
                        nc.tensor.matmul(sc_lo[:, :512], lhs, rl[:, :512],
                                         start=(b == 0 and ac == 0),
                                         stop=(b == BL - 1 and ac == 1))
                        nc.tensor.matmul(sc_hi[:, :64], lhs, rl[:, 512:L],
                                         start=(b == 0 and ac == 0),
                                         stop=(b == BL - 1 and ac == 1))

                # ---- softmax over L ----
                r1 = sb.tile([BL, 1], f32, tag="r1")
                r2 = sb.tile([BL, 1], f32, tag="r2")
                nc.vector.tensor_reduce(r1[:], sc_lo[:, :512],
                                        mybir.AxisListType.X, OP.max)
                nc.vector.tensor_reduce(r2[:], sc_hi[:, :64],
                                        mybir.AxisListType.X, OP.max)
                mneg = sb.tile([BL, 1], f32, tag="mneg")
                nc.vector.tensor_tensor(mneg[:], r1[:], r2[:], op=OP.max)
                nc.vector.tensor_scalar(mneg[:], mneg[:], -1.0, None, OP.mult)
                alpha = sb.tile([BL, L], f32, tag="alpha")
                ssum1 = sb.tile([BL, 1], f32, tag="ssum1")
                ssum2 = sb.tile([BL, 1], f32, tag="ssum2")
                nc.scalar.activation(alpha[:, :512], sc_lo[:, :512], AF.Exp,
                                     bias=mneg[:, :1], accum_out=ssum1[:])
                nc.scalar.activation(alpha[:, 512:L], sc_hi[:, :64], AF.Exp,
                                     bias=mneg[:, :1], accum_out=ssum2[:])
                ssum = sb.tile([BL, 1], f32, tag="ssum")
                nc.vector.tensor_tensor(ssum[:], ssum1[:], ssum2[:], op=OP.add)
                rec = sb.tile([BL, 1], f32, tag="rec")
                nc.vector.reciprocal(rec[:], ssum[:])
                nc.vector.tensor_scalar(alpha[:], alpha[:], rec[:, :1], None,
                                        OP.mult)
                # alphaT fp16 [128, 5*16]
                alphaT = sb.tile([128, LC * BL], f16, tag="alphaT")
                for lc in range(LC):
                    n = LCS[lc]
                    tt = tpp.tile([128, BL], f32, tag="tp")
                    nc.tensor.transpose(tt[:n, :BL],
                                        alpha[:, lc * 128:lc * 128 + n],
                                        ident[:BL, :BL])
                    nc.vector.tensor_copy(alphaT[:n, lc * BL:(lc + 1) * BL],
                                          tt[:n, :BL])
                # zero-padded diag tile [128, 5*16*16]
                pad = sb.tile([128, LC * BL * BL], f16, tag="pad")
                nc.vector.memset(pad[:], 0.0)
                pad_diag = bass.AP(pad.tensor, pad.offset,
                                   [pad.ap[0], [BL * BL, LC], [BL + 1, BL]])
                src_a = bass.AP(alphaT.tensor, alphaT.offset,
                                [alphaT.ap[0], [BL, LC], [1, BL]])
                nc.vector.tensor_copy(pad_diag, src_a)

                # ---- awe: accumulate [16, 512] x 2 over l chunks ----
                awe0 = ps.tile([BL, 512], f32, tag="ps")
                awe1 = ps.tile([BL, 512], f32, tag="ps")
                for lc in range(LC):
                    n = LCS[lc]
                    for bg in range(4):
                        slab = memp.tile([128, 4 * Denc], f16, tag="mem")
                        nc.sync.dma_start(
                            slab[:n, :],
                            mem16_d[lc, :n, bg * 4 * Denc:(bg + 1) * 4 * Denc])
                        for bb in range(4):
                            b = bg * 4 + bb
                            lhs = pad[:n, (lc * BL + b) * BL:(lc * BL + b + 1) * BL]
                            nc.tensor.matmul(
                                awe0[:, :512], lhs,
                                slab[:n, bb * Denc:bb * Denc + 512],
                                start=(lc == 0 and b == 0),
                                stop=(lc == LC - 1 and b == BL - 1))
                            nc.tensor.matmul(
                                awe1[:, :512], lhs,
                                slab[:n, bb * Denc + 512:(bb + 1) * Denc],
                                start=(lc == 0 and b == 0),
                                stop=(lc == LC - 1 and b == BL - 1))

                # ---- gate = sigmoid(h1 @ Wfb + bfb) ----
                gt0 = ps.tile([BL, 512], f32, tag="ps")
                gt1 = ps.tile([BL, 512], f32, tag="ps")
                for k in range(4):
                    lhs = xT[:, (10 + k) * BL:(11 + k) * BL]
                    nc.tensor.matmul(gt0[:, :512], lhs,
                                     Wfb[:, k * Denc:k * Denc + 512],
                                     start=(k == 0), stop=(k == 3))
                    nc.tensor.matmul(gt1[:, :512], lhs,
                                     Wfb[:, k * Denc + 512:(k + 1) * Denc],
                                     start=(k == 0), stop=(k == 3))
                gpre = sb.tile([BL, Denc], f32, tag="gpre")
                nc.vector.tensor_tensor(gpre[:, :512], gt0[:, :512],
                                        bfbr[:, :512], op=OP.add)
                nc.vector.tensor_tensor(gpre[:, 512:], gt1[:, :512],
                                        bfbr[:, 512:], op=OP.add)
                gate = sb.tile([BL, Denc], f32, tag="gate")
                nc.scalar.activation(gate[:], gpre[:], AF.Sigmoid)
                ga = sb.tile([BL, Denc], f32, tag="ga")
                nc.vector.tensor_tensor(ga[:, :512], gate[:, :512],
                                        awe0[:, :512], op=OP.mult)
                nc.vector.tensor_tensor(ga[:, 512:], gate[:, 512:],
                                        awe1[:, :512], op=OP.mult)
                # xT chunks 2..9 from ga
                for k in range(8):
                    transpose_into(xT[:, (2 + k) * BL:(3 + k) * BL],
                                   ga[:, k * 128:(k + 1) * 128])

                # ---- LSTM1: g = x @ W1 (streamed) + b1 ----
                g_ps = []
                for _q in range(4):
                    gq = ps.tile([BL, 512], f32, tag="ps")
                    g_ps.append(gq)
                for k in range(KX):
                    wt = w1p.tile([128, 4 * Dh], f32, tag="w1")
                    nc.sync.dma_start(wt[:], W1_d[k])
                    for q in range(4):
                        nc.tensor.matmul(g_ps[q][:, :512],
                                         xT[:, k * BL:(k + 1) * BL],
                                         wt[:, q * 512:(q + 1) * 512],
                                         start=(k == 0), stop=(k == KX - 1))

                def lstm_cell(g_ps, br, c_t, h_t):
                    pre = sb.tile([BL, 4 * Dh], f32, tag="pre")
                    for q in range(4):
                        nc.vector.tensor_tensor(pre[:, q * 512:(q + 1) * 512],
                                                g_ps[q][:, :512],
                                                br[:, q * 512:(q + 1) * 512],
                                                op=OP.add)
                    act = sb.tile([BL, 4 * Dh], f32, tag="act")
                    nc.scalar.activation(act[:, :512], pre[:, :512], AF.Sigmoid)
                    nc.scalar.activation(act[:, 512:1024], pre[:, 512:1024],
                                         AF.Sigmoid)
                    nc.scalar.activation(act[:, 1024:1536], pre[:, 1024:1536],
                                         AF.Tanh)
                    nc.scalar.activation(act[:, 1536:], pre[:, 1536:],
                                         AF.Sigmoid)
                    t1 = sb.tile([BL, Dh], f32, tag="t1")
                    nc.vector.tensor_tensor(t1[:], act[:, 512:1024], c_t[:],
                                            op=OP.mult)
                    t2 = sb.tile([BL, Dh], f32, tag="t2")
                    nc.vector.tensor_tensor(t2[:], act[:, :512],
                                            act[:, 1024:1536], op=OP.mult)
                    nc.vector.tensor_tensor(c_t[:], t1[:], t2[:], op=OP.add)
                    tc_ = sb.tile([BL, Dh], f32, tag="tc")
                    nc.scalar.activation(tc_[:], c_t[:], AF.Tanh)
                    nc.vector.tensor_tensor(h_t[:], act[:, 1536:], tc_[:],
                                            op=OP.mult)

                lstm_cell(g_ps, b1r, c1, h1)
                # refresh h1T chunks in xT
                for k in range(4):
                    transpose_into(xT[:, (10 + k) * BL:(11 + k) * BL],
                                   h1[:, k * 128:(k + 1) * 128])

                # ---- LSTM2: g2 = h1 @ Wih2 + h2 @ Whh2 + b2 ----
                g2_ps = []
                for _q in range(4):
                    g2q = ps.tile([BL, 512], f32, tag="ps")
                    g2_ps.append(g2q)
                for k in range(4):
                    wt2 = w1p.tile([128, 4 * Dh], f32, tag="w1")
                    nc.sync.dma_start(wt2[:], W2_d[k])
                    for q in range(4):
                        nc.tensor.matmul(
                            g2_ps[q][:, :512],
                            xT[:, (10 + k) * BL:(11 + k) * BL],
                            wt2[:, q * 512:(q + 1) * 512],
                            start=(k == 0), stop=False)
                for k in range(4):
                    wt2 = w1p.tile([128, 4 * Dh], f32, tag="w1")
                    nc.sync.dma_start(wt2[:], W2_d[4 + k])
                    for q in range(4):
                        nc.tensor.matmul(
                            g2_ps[q][:, :512],
                            h2T[:, k * BL:(k + 1) * BL],
                            wt2[:, q * 512:(q + 1) * 512],
                            start=False, stop=(k == 3))
                lstm_cell(g2_ps, b2r, c2, h2)
                for k in range(4):
                    transpose_into(h2T[:, k * BL:(k + 1) * BL],
                                   h2[:, k * 128:(k + 1) * 128])

                # ---- FC: logits = h2 @ Wfc + bfc ----
                lg0 = ps.tile([BL, 512], f32, tag="ps")
                lg1 = ps.tile([BL, 512], f32, tag="ps")
                for k in range(4):
                    wtf = w1p.tile([128, 4 * Dh], f32, tag="w1")
                    nc.sync.dma_start(wtf[:, :V], Wfc_d[k])
                    lhs = h2T[:, k * BL:(k + 1) * BL]
                    nc.tensor.matmul(lg0[:, :500], lhs,
                                     wtf[:, :500],
                                     start=(k == 0), stop=(k == 3))
                    nc.tensor.matmul(lg1[:, :500], lhs,
                                     wtf[:, 500:V],
                                     start=(k == 0), stop=(k == 3))
                logit = sb.tile([BL, V], f32, tag="logit")
                nc.vector.tensor_tensor(logit[:, :500], lg0[:, :500],
                                        bfcr[:, :500], op=OP.add)
                nc.vector.tensor_tensor(logit[:, 500:], lg1[:, :500],
                                        bfcr[:, 500:], op=OP.add)

                # ---- exchange with pair core ----
                bin_ = dram.tile([BL, V], f32, tag="bin")
                bout = dram.tile([2 * BL, V], f32, tag="bout")
                nc.sync.dma_start(bin_[:], logit[:])
                nc.gpsimd.collective_compute(
                    "AllGather", OP.bypass, replica_groups=GROUPS,
                    ins=[bin_[:].opt()], outs=[bout[:].opt()])
                l0 = sb.tile([BL, V], f32, tag="l0")
                l1 = sb.tile([BL, V], f32, tag="l1")
                nc.sync.dma_start(l0[:], bout[:BL, :])
                nc.sync.dma_start(l1[:], bout[BL:, :])

                # stats for host-side score reconstruction
                scr = sb.tile([BL, V], f32, tag="scr")
                nc.vector.tensor_reduce(m0_t[:, t:t + 1], l0[:],
                                        mybir.AxisListType.X, OP.max,
                                        negate=True)
                nc.vector.tensor_reduce(m1_t[:, t:t + 1], l1[:],
                                        mybir.AxisListType.X, OP.max,
                                        negate=True)
                nc.scalar.activation(scr[:], l0[:], AF.Exp,
                                     bias=m0_t[:, t:t + 1],
                                     accum_out=s0_t[:, t:t + 1])
                nc.scalar.activation(scr[:], l1[:], AF.Exp,
                                     bias=m1_t[:, t:t + 1],
                                     accum_out=s1_t[:, t:t + 1])
                lsum = sb.tile([BL, V], f32, tag="lsum")
                nc.vector.tensor_tensor(lsum[:], l0[:], l1[:], op=OP.add)
                maxv = sb.tile([BL, 1], f32, tag="maxv")
                nc.vector.tensor_reduce(maxv[:], lsum[:],
                                        mybir.AxisListType.X, OP.max)
                mask = sb.tile([BL, V], i32, tag="mask")
                nc.vector.tensor_scalar(mask[:], lsum[:], maxv[:, :1], None,
                                        OP.is_equal)
                cand = sb.tile([BL, V], f32, tag="cand")
                nc.vector.memset(cand[:], BIG)
                nc.vector.copy_predicated(cand[:], mask[:], iotar[:])
                nxt = sb.tile([BL, 1], f32, tag="nxt")
                nc.vector.tensor_reduce(nxt[:], cand[:],
                                        mybir.AxisListType.X, OP.min)
                # l0v/l1v at nxt
                sel = sb.tile([BL, V], f32, tag="sel")
                nc.vector.tensor_scalar(sel[:], iotar[:], nxt[:, :1], None,
                                        OP.is_equal)
                pk = sb.tile([BL, V], f32, tag="pk")
                nc.vector.tensor_tensor(pk[:], sel[:], l0[:], op=OP.mult)
                nc.vector.tensor_reduce(l0v_t[:, t:t + 1], pk[:],
                                        mybir.AxisListType.X, OP.add)
                pk2 = sb.tile([BL, V], f32, tag="pk2")
                nc.vector.tensor_tensor(pk2[:], sel[:], l1[:], op=OP.mult)
                nc.vector.tensor_reduce(l1v_t[:, t:t + 1], pk2[:],
                                        mybir.AxisListType.X, OP.add)
                # token masking and finished update
                omf = sb.tile([BL, 1], f32, tag="omf")
                nc.vector.tensor_scalar(omf[:], fin[:], -1.0, 1.0, OP.mult,
                                        OP.add)
                nxtm = sb.tile([BL, 1], f32, tag="nxtm")
                nc.vector.tensor_tensor(nxtm[:], nxt[:], omf[:], op=OP.mult)
                eosq = sb.tile([BL, 1], f32, tag="eosq")
                nc.vector.tensor_scalar(eosq[:], nxtm[:], float(EOS), None,
                                        OP.is_equal)
                nc.vector.tensor_tensor(fin[:], fin[:], eosq[:], op=OP.max)
                nc.vector.tensor_copy(preds_t[:, t:t + 1], nxtm[:])
                # next embedding
                toki2 = sb.tile([BL, 1], i32, tag="toki2")
                nc.vector.tensor_copy(toki2[:], nxtm[:])
                e_sb2 = sb.tile([BL, Demb], f32, tag="e_sb2")
                nc.gpsimd.indirect_dma_start(
                    out=e_sb2[:], out_offset=None, in_=emb_d[:],
                    in_offset=bass.IndirectOffsetOnAxis(ap=toki2[:, :1],
                                                        axis=0))
                for k in range(2):
                    transpose_into(xT[:, k * BL:(k + 1) * BL],
                                   e_sb2[:, k * 128:(k + 1) * 128])
                if t == NSTEP - 1:
                    nc.vector.tensor_copy(tokf0[:], nxtm[:])

            # ---- write back state & stats ----
            nc.sync.dma_start(h1_o[:], h1[:])
            nc.sync.dma_start(c1_o[:], c1[:])
            nc.sync.dma_start(h2_o[:], h2[:])
            nc.sync.dma_start(c2_o[:], c2[:])
            nc.sync.dma_start(tok_o[:], tokf0[:])
            nc.sync.dma_start(fin_o[:], fin[:])
            nc.sync.dma_start(preds_o[:], preds_t[:])
            nc.sync.dma_start(m0_o[:], m0_t[:])
            nc.sync.dma_start(m1_o[:], m1_t[:])
            nc.sync.dma_start(s0_o[:], s0_t[:])
            nc.sync.dma_start(s1_o[:], s1_t[:])
            nc.sync.dma_start(l0v_o[:], l0v_t[:])
            nc.sync.dma_start(l1v_o[:], l1v_t[:])

    nc.compile()
    return nc


def kernel(memory, emb, W_att_enc, W_att_dec, v_att, W_fbeta, b_fbeta,
           W_ih1, W_hh1, b1, W_ih2, W_hh2, b2, W_fc, b_fc, W_init_h, W_init_c):
    global _compiled
    from concourse.bass_utils import run_bass_kernel_spmd
    import time

    memory = np.asarray(memory, np.float32)
    # ---- host setup (one-time, fp32 exact like the reference) ----
    # att1 = memory @ W_att_enc ; mean/init state
    att1 = np.einsum('ebld,eda->ebla', memory, W_att_enc,
                     optimize=True).astype(np.float32)
    mean_mem = memory.mean(axis=2)
    h1_0 = np.tanh(np.einsum('ebd,edh->ebh', mean_mem, W_init_h)).astype(np.float32)
    c1_0 = np.tanh(np.einsum('ebd,edh->ebh', mean_mem, W_init_c)).astype(np.float32)

    if _compiled is None:
        _compiled = _build()
    nc = _compiled

    # ---- per-core static inputs ----
    pad128 = np.zeros((E, BL * 4, 128, L), np.float16)
    core_static = []
    for c in range(NCORES):
        e, s = c // 4, c % 4
        bs = s * BL
        m = memory[e, bs:bs + BL]                       # [16, 576, 1024]
        m16 = m.astype(np.float16)
        mem16 = np.zeros((LC, 128, BL * Denc), np.float16)
        for lc in range(LC):
            n = LCS[lc]
            # [n, b, d] from [b, l, d]
            mem16[lc, :n] = m16[:, lc * 128:lc * 128 + n, :].transpose(1, 0, 2) \
                .reshape(n, BL * Denc)
        a1 = att1[e, bs:bs + BL]                        # [16, 576, 256]
        a1T = a1.transpose(0, 2, 1).astype(np.float16)  # [16, 256, 576]
        att1T = a1T.reshape(BL, 2, 128, L).reshape(BL * 2, 128, L)
        W1 = np.concatenate([W_ih1[e], W_hh1[e]], axis=0)   # [1792, 2048]
        W1 = np.ascontiguousarray(W1.reshape(KX, 128, 4 * Dh), np.float32)
        W2 = np.concatenate([W_ih2[e], W_hh2[e]], axis=0)   # [1024, 2048]
        W2 = np.ascontiguousarray(W2.reshape(8, 128, 4 * Dh), np.float32)
        Wfc = np.ascontiguousarray(W_fc[e].reshape(4, 128, V), np.float32)
        Wdec = np.ascontiguousarray(W_att_dec[e].reshape(4, 128, Datt), np.float32)
        Wfb = np.ascontiguousarray(W_fbeta[e].reshape(4, 128, Denc), np.float32)
        v16 = v_att[e].astype(np.float16)
        vpad = np.zeros((128, 2 * BL * BL), np.float16)
        for ac in range(2):
            for b in range(BL):
                vpad[:, (ac * BL + b) * BL + b] = v16[ac * 128:(ac + 1) * 128]
        iota = np.tile(np.arange(V, dtype=np.float32), (BL, 1))
        core_static.append(dict(
            mem16=mem16, att1T=att1T, embw=np.asarray(emb[e], np.float32),
            W1=W1, W2=W2, Wfc=Wfc, Wdec=Wdec, Wfb=Wfb, vpad=vpad,
            b1r=np.tile(b1[e], (BL, 1)).astype(np.float32),
            b2r=np.tile(b2[e], (BL, 1)).astype(np.float32),
            bfbr=np.tile(b_fbeta[e], (BL, 1)).astype(np.float32),
            bfcr=np.tile(b_fc[e], (BL, 1)).astype(np.float32),
            iotar=iota,
            ident=np.eye(128, dtype=np.float32),
        ))

    # ---- run chunks ----
    state = []
    for c in range(NCORES):
        e, s = c // 4, c % 4
        bs = s * BL
        state.append(dict(
            h1_in=h1_0[e, bs:bs + BL].copy(), c1_in=c1_0[e, bs:bs + BL].copy(),
            h2_in=np.zeros((BL, Dh), np.float32),
            c2_in=np.zeros((BL, Dh), np.float32),
            tok_in=np.full((BL, 1), float(SOS), np.float32),
            fin_in=np.zeros((BL, 1), np.float32),
        ))

    nchunks = MAX_LEN // NSTEP
    all_stats = []
    t_exec = 0.0
    call_times = []
    for ck in range(nchunks):
        in_maps = [{**core_static[c], **state[c]} for c in range(NCORES)]
        t0 = time.perf_counter()
        res = run_bass_kernel_spmd(nc, in_maps, core_ids=list(range(NCORES)))
        call_times.append(time.perf_counter() - t0)
        t_exec += call_times[-1]
        outs = res.results
        all_stats.append([outs[c] for c in range(NCORES)])
        for c in range(NCORES):
            o = outs[c]
            state[c] = dict(
                h1_in=o["h1_out"], c1_in=o["c1_out"],
                h2_in=o["h2_out"], c2_in=o["c2_out"],
                tok_in=o["tok_out"], fin_in=o["fin_out"])
    kernel.last_exec_wall = t_exec
    kernel.last_call_times = call_times

    # ---- assemble outputs ----
    preds = np.zeros((B, MAX_LEN), np.int32)
    score = np.zeros((B,), np.float32)
    fin_h = np.zeros((B,), bool)
    for ck in range(nchunks):
        for s in range(4):
            o = all_stats[ck][s]           # member-0 group core for shard s
            bs = s * BL
            p = o["preds"]                 # [16, NSTEP] fp32
            preds[bs:bs + BL, ck * NSTEP:(ck + 1) * NSTEP] = p.astype(np.int32)
            # score reconstruction (exact fp32 order of reference)
            m0 = o["m0s"]; m1 = o["m1s"]   # -max
            s0 = o["s0s"]; s1 = o["s1s"]
            l0v = o["l0vs"]; l1v = o["l1vs"]
            for t in range(NSTEP):
                lp0 = (l0v[:, t] + m0[:, t]).astype(np.float32) - \
                    np.log(s0[:, t]).astype(np.float32)
                lp1 = (l1v[:, t] + m1[:, t]).astype(np.float32) - \
                    np.log(s1[:, t]).astype(np.float32)
                lp = ((lp0 + lp1) * np.float32(0.5)).astype(np.float32)
                score[bs:bs + BL] += np.where(fin_h[bs:bs + BL],
                                              np.float32(0), lp)
                tok = preds[bs:bs + BL, ck * NSTEP + t]
                fin_h[bs:bs + BL] |= (tok == EOS)
    return preds, score
